# revision 1
# baseline (speedup 1.0000x reference)
"""Trainium2 Bass kernel for nn_CropAndPadMask (paste instance masks into canvases).

Math: for each (b, n) box the reference output is a bilinear resize of the
28x28 mask pasted into a zero [H, W] canvas.  Bilinear resize + paste is a
pair of small matmuls:

    out[b, n] = Wy[b, n] @ mask[b, n] @ Wx[b, n]

with Wy [H, 28] holding the y-interpolation weights (box/validity mask folded
in) and Wx [28, W] the x-interpolation weights.  The host precomputes these
tiny weight matrices from det_outs; the device does the heavy part: 5 matmuls
per canvas on TensorE and streams the 256 MiB of canvases out to HBM.

Sharding: 256 canvases are split 32-per-core across the 8 NeuronCores
(pure data parallel, no communication).
"""

import sys

for _p in ("/opt/trn_rl_repo", "/root/.axon_site/_ro/trn_rl_repo"):
    if _p not in sys.path:
        sys.path.append(_p)

import numpy as np

import concourse.bass as bass
import concourse.mybir as mybir
import concourse.tile as tile
from concourse.bass_utils import run_bass_kernel_spmd

B, N, H, W, MH, MW = 4, 64, 512, 512, 28, 28
N_CORES = 8
CPC = (B * N) // N_CORES  # canvases per core = 32
G = 2                     # canvases per output DMA group
KCH = 4                   # row chunks per canvas (H = KCH * 128)
FREE = 2 * W + MW         # per-canvas weight row: wyT | wx | maskT

def _split_multi_waits(nc: bass.Bass) -> None:
    """The walrus bundled in this container accepts at most ONE sync wait per
    instruction.  Tile freely attaches several.  Hoist the extras onto
    standalone EventSemaphore carriers inserted just before the instruction on
    the same engine (per-engine program order makes this equivalent)."""
    n_new = 0
    for f in nc.m.functions:
        for bb in f.blocks:
            lst = bb.instructions
            i = 0
            while i < len(lst):
                ins = lst[i]
                si = ins.sync_info
                if si is not None and si.on_wait and len(si.on_wait) > 1:
                    waits = list(si.on_wait)
                    ins.sync_info = mybir.SyncInfo(
                        on_wait=waits[:1], on_update=list(si.on_update or [])
                    )
                    carriers = []
                    for w in waits[1:]:
                        n_new += 1
                        carriers.append(
                            mybir.InstEventSemaphore(
                                name=f"I-waitsplit-{n_new}",
                                ins=[],
                                outs=[],
                                engine=ins.engine,
                                sync_info=mybir.SyncInfo(on_wait=[w], on_update=[]),
                            )
                        )
                    lst[i:i] = carriers
                    i += len(carriers)
                i += 1


def build_nc(cpc: int = CPC, g: int = G) -> bass.Bass:
    """One core's program: cpc canvases, streamed out g at a time."""
    f32 = mybir.dt.float32
    f32r = mybir.dt.float32r
    nc = bass.Bass()
    wmat = nc.dram_tensor("wmat", [cpc, MW, FREE], f32r, kind="ExternalInput")
    out = nc.dram_tensor("out", [cpc, H, W], f32, kind="ExternalOutput")

    PREFETCH = 4  # input DMAs issued this many canvases ahead

    with tile.TileContext(nc) as tc:
        with (
            tc.tile_pool(name="win", bufs=PREFETCH + 3) as win_pool,
            tc.tile_pool(name="ssb", bufs=4) as s_pool,
            tc.tile_pool(name="stage", bufs=4) as stage_pool,
            tc.tile_pool(name="psum_s", bufs=2, space="PSUM") as psum_s,
            tc.tile_pool(name="psum_c", bufs=6, space="PSUM") as psum_c,
        ):
            w_tiles: dict[int, object] = {}
            s_tiles: dict[int, object] = {}
            n_cp = 0

            def load_w(c):
                if c < cpc and c not in w_tiles:
                    w_t = win_pool.tile([MW, FREE], f32r)
                    nc.gpsimd.dma_start(w_t[:], wmat[c])
                    w_tiles[c] = w_t

            def mm1_and_scopy(c):
                """S = mask @ Wx : lhsT = mask^T [MW, MH], rhs = Wx [MW, W]"""
                nonlocal n_cp
                if c >= cpc or c in s_tiles:
                    return
                w_t = w_tiles[c]
                s_p = psum_s.tile([MH, W], f32)
                nc.tensor.matmul(
                    s_p[:], w_t[:, 2 * W :], w_t[:, W : 2 * W],
                    start=True, stop=True,
                )
                s_sb = s_pool.tile([MH, W], f32r)
                if n_cp % 2 == 0:
                    nc.scalar.copy(s_sb[:], s_p[:])
                else:
                    nc.vector.tensor_copy(s_sb[:], s_p[:])
                n_cp += 1
                s_tiles[c] = s_sb

            for c in range(min(PREFETCH, cpc)):
                load_w(c)
            mm1_and_scopy(0)

            # small first/last groups: the first out-DMA starts after one
            # canvas (shorter pipeline fill) and the final drain is 1 MiB
            if cpc >= 8:
                group_sizes = [1, 1] + [g] * ((cpc - 4) // g) + [1, 1]
            else:
                group_sizes = [g] * (cpc // g)
            assert sum(group_sizes) == cpc
            c0 = 0
            for gsz in group_sizes:
                stage = stage_pool.tile([128, g, KCH, W], f32, tag="stage")
                for cc in range(gsz):
                    c = c0 + cc
                    load_w(c + PREFETCH)
                    # pipeline: next canvas's S is produced while this one's
                    # chunk matmuls stream, so PE never waits on the S-copy
                    mm1_and_scopy(c + 1)
                    w_t, s_sb = w_tiles.pop(c), s_tiles.pop(c)
                    for k in range(KCH):
                        # rows {k, k+4, ...} of the canvas: lhsT = WyT cols k::4
                        p_k = psum_c.tile([128, W], f32)
                        nc.tensor.matmul(
                            p_k[:], w_t[:, 128 * k : 128 * (k + 1)], s_sb[:],
                            start=True, stop=True,
                        )
                        if n_cp % 2 == 0:
                            nc.scalar.copy(stage[:, cc, k, :], p_k[:])
                        else:
                            nc.vector.tensor_copy(stage[:, cc, k, :], p_k[:])
                        n_cp += 1
                # canvas row = 4*p + k  ->  DRAM view [p, c, k, w]
                out_ap = out[c0 : c0 + gsz].rearrange("c (p k) w -> p c k w", k=KCH)
                nc.sync.dma_start(out_ap, stage[:, :gsz])
                c0 += gsz
    _split_multi_waits(nc)
    return nc


def _box_weight_matrices(det_outs: np.ndarray) -> tuple[np.ndarray, np.ndarray]:
    """Wy [BN, H, MH] and Wx [BN, MW, W] (f32), reference semantics."""
    det = np.asarray(det_outs, dtype=np.float32).reshape(B * N, 6)
    score = det[:, 5]
    thr = np.float32(50.0) if np.max(score) > 50.0 else np.float32(-100.0)
    valid = score >= thr
    box = np.maximum(det, np.float32(1.0))
    cx, cy, w, h = box[:, 0], box[:, 1], box[:, 2], box[:, 3]
    two = np.float32(2.0)
    xmin = np.clip(np.ceil(cx - w / two).astype(np.int32), 0, W)
    xmax = np.clip(np.ceil(cx + w / two).astype(np.int32), 0, W)
    ymin = np.clip(np.ceil(cy - h / two).astype(np.int32), 0, H)
    ymax = np.clip(np.ceil(cy + h / two).astype(np.int32), 0, H)
    out_h = (ymax - ymin).astype(np.float32)
    out_w = (xmax - xmin).astype(np.float32)
    one = np.float32(1.0)
    sy = np.where(out_h > one, np.float32(MH - 1) / np.maximum(out_h - one, one),
                  np.float32(0.0)).astype(np.float32)
    sx = np.where(out_w > one, np.float32(MW - 1) / np.maximum(out_w - one, one),
                  np.float32(0.0)).astype(np.float32)

    ys = np.arange(H, dtype=np.float32)
    xs = np.arange(W, dtype=np.float32)
    src_y = (ys[None, :] - ymin[:, None].astype(np.float32)) * sy[:, None]
    src_x = (xs[None, :] - xmin[:, None].astype(np.float32)) * sx[:, None]
    src_y = np.clip(src_y, np.float32(0.0), np.float32(MH - 1)).astype(np.float32)
    src_x = np.clip(src_x, np.float32(0.0), np.float32(MW - 1)).astype(np.float32)

    y0 = np.floor(src_y).astype(np.int32)
    y1 = np.minimum(y0 + 1, MH - 1)
    wy = (src_y - y0.astype(np.float32)).astype(np.float32)
    x0 = np.floor(src_x).astype(np.int32)
    x1 = np.minimum(x0 + 1, MW - 1)
    wx = (src_x - x0.astype(np.float32)).astype(np.float32)

    keep_y = ((ys[None, :] >= ymin[:, None].astype(np.float32))
              & (ys[None, :] < ymax[:, None].astype(np.float32))
              & valid[:, None]).astype(np.float32)
    keep_x = ((xs[None, :] >= xmin[:, None].astype(np.float32))
              & (xs[None, :] < xmax[:, None].astype(np.float32))).astype(np.float32)

    m = np.arange(MH, dtype=np.int32)
    Wy = ((m[None, None, :] == y0[:, :, None]) * (one - wy[:, :, None])
          + (m[None, None, :] == y1[:, :, None]) * wy[:, :, None]).astype(np.float32)
    Wy *= keep_y[:, :, None]
    Wx = ((m[None, :, None] == x0[:, None, :]) * (one - wx[:, None, :])
          + (m[None, :, None] == x1[:, None, :]) * wx[:, None, :]).astype(np.float32)
    Wx *= keep_x[:, None, :]
    return Wy, Wx


_ROW_PERM = np.concatenate([KCH * np.arange(H // KCH) + k for k in range(KCH)])


def prepare_in_maps(det_outs: np.ndarray, ins_outs: np.ndarray,
                    cpc: int = CPC, n_cores: int = N_CORES) -> list[dict]:
    Wy, Wx = _box_weight_matrices(det_outs)
    # wyT [BN, MH, H], columns permuted so block k holds rows k::KCH
    wyT = np.ascontiguousarray(np.transpose(Wy, (0, 2, 1)))[:, :, _ROW_PERM]
    masksT = np.ascontiguousarray(
        np.transpose(np.asarray(ins_outs, np.float32).reshape(B * N, MH, MW),
                     (0, 2, 1)))
    wmat = np.concatenate([wyT, Wx, masksT], axis=2).astype(np.float32)
    assert wmat.shape == (B * N, MW, FREE)
    return [{"wmat": np.ascontiguousarray(wmat[i * cpc : (i + 1) * cpc])}
            for i in range(n_cores)]


def kernel(images: np.ndarray, det_outs: np.ndarray, ins_outs: np.ndarray) -> np.ndarray:
    nc = build_nc()
    in_maps = prepare_in_maps(det_outs, ins_outs)
    res = run_bass_kernel_spmd(nc, in_maps, list(range(N_CORES)))
    full = np.concatenate([res.results[i]["out"] for i in range(N_CORES)], axis=0)
    return full.reshape(B, N, H, W).astype(np.float32)



# revision 7
# speedup vs baseline: 3.4643x; 3.4643x over previous
"""Trainium2 Bass kernel for nn_CropAndPadMask (paste instance masks into canvases).

Math: for each (b, n) box the reference output is a bilinear resize of the
28x28 mask pasted into a zero [H, W] canvas.  Every non-zero output value
lies inside the box window [ymin, ymax) x [xmin, xmax), whose extent is at
most 200x200 (box w, h <= 200).  So instead of streaming 256 MiB of mostly
zero canvases, each core computes a fixed 200x200 f16 patch per box:

    patch[b, n] = WyP[b, n] @ mask[b, n] @ WxP[b, n]

with WyP [200, 28] the y-interpolation weights for rows ymin..ymin+199
(validity/box mask folded in) and WxP [28, 200] the x-weights for cols
xmin..xmin+199.  The host pastes the patches into a zero canvas while
gathering (pure data movement), which drops device HBM traffic ~10x.

Device layout details:
- Boxes are processed in pairs: the two S = mask @ WxP matmuls of a pair
  land at PE tile positions (0,0)/(32,32) in ONE [64, 200] PSUM tile so a
  single PSUM->SBUF copy serves both (operand base partitions are limited
  to {0, 32, 64} by the IR, so 4-way stacking at 96 is not available).
- The final patch is computed as two [100, 200] matmuls (even / odd canvas
  rows) into one [100, 400] PSUM tile: partition p holds rows 2p and 2p+1,
  making DRAM descriptors 800 B (full DMA-engine rate at f16).
- f16 everywhere off-PSUM: PE runs at 1 cycle/row (f32r would be 4x slower
  at these free sizes) and DMA bytes halve.  abs values <= 1 so f16 rounding
  (~5e-4 rel) is far inside the 2e-2 gate.

Sharding: 256 boxes split 32-per-core across 8 NeuronCores (pure data
parallel, no communication).
"""

import sys

for _p in ("/opt/trn_rl_repo", "/root/.axon_site/_ro/trn_rl_repo"):
    if _p not in sys.path:
        sys.path.append(_p)

import numpy as np

import concourse.bass as bass
import concourse.mybir as mybir
import concourse.tile as tile
from concourse.bass_utils import run_bass_kernel_spmd

B, N, H, W, MH, MW = 4, 64, 512, 512, 28, 28
N_CORES = 8
CPC = (B * N) // N_CORES  # canvases per core = 32
PH = PW = 200             # patch extent (boxes are <= 200x200)
GQ = 4                    # boxes per group (stacked S + one out-DMA)
NG = CPC // GQ            # groups per core = 8
# per-box input row (28 partitions used, padded to 32):
#   WxP [28, 200] | maskT [28, 32] | wyT_even [28, 100] | wyT_odd [28, 100]
FREE = PW + 32 + PH
OUT_F = 2 * PW            # patch stored [100, 2*200]: partition p = rows 2p, 2p+1


def _split_multi_waits(nc: bass.Bass) -> None:
    """The walrus bundled in this container accepts at most ONE sync wait per
    instruction.  Tile freely attaches several.  Hoist the extras onto
    standalone EventSemaphore carriers inserted just before the instruction on
    the same engine (per-engine program order makes this equivalent)."""
    n_new = 0
    for f in nc.m.functions:
        for bb in f.blocks:
            lst = bb.instructions
            i = 0
            while i < len(lst):
                ins = lst[i]
                si = ins.sync_info
                if si is not None and si.on_wait and len(si.on_wait) > 1:
                    waits = list(si.on_wait)
                    ins.sync_info = mybir.SyncInfo(
                        on_wait=waits[:1], on_update=list(si.on_update or [])
                    )
                    carriers = []
                    for w in waits[1:]:
                        n_new += 1
                        carriers.append(
                            mybir.InstEventSemaphore(
                                name=f"I-waitsplit-{n_new}",
                                ins=[],
                                outs=[],
                                engine=ins.engine,
                                sync_info=mybir.SyncInfo(on_wait=[w], on_update=[]),
                            )
                        )
                    lst[i:i] = carriers
                    i += len(carriers)
                i += 1


def build_nc(cpc: int = CPC) -> bass.Bass:
    """One core's program: cpc boxes, 4 per group."""
    f16 = mybir.dt.float16
    f32 = mybir.dt.float32
    ng = cpc // GQ
    nc = bass.Bass()
    wmat = nc.dram_tensor("wmat", [cpc, 32, FREE], f16, kind="ExternalInput")
    out = nc.dram_tensor("out", [cpc, PH // 2, OUT_F], f16, kind="ExternalOutput")

    with tile.TileContext(nc) as tc:
        with (
            tc.tile_pool(name="win", bufs=3) as win_pool,
            tc.tile_pool(name="ssb", bufs=4) as s_pool,
            tc.tile_pool(name="stage", bufs=3) as stage_pool,
            tc.tile_pool(name="psum_s", bufs=4, space="PSUM") as psum_s,
            tc.tile_pool(name="psum_c", bufs=4, space="PSUM") as psum_c,
        ):
            # ---- input loads: group 0 via sync HWDGE (fast start), groups
            # 1-2 via scalar HWDGE, the rest via gpsimd SWDGE (one big DMA).
            # Boxes live in pairs: box c = (pair j = c//2, e = c%2), SBUF
            # partition 32*e + p.
            npairs = cpc // 2
            load_plan = [
                (0, 2, "sync"),
                (2, 4, "scalar"),
                (6, npairs - 6, "gpsimd"),
            ] if ng > 3 else [(2 * g, 2, "sync") for g in range(ng)]
            win_of: dict[int, tuple[object, int]] = {}
            for j0, jlen, eng in load_plan:
                w_t = win_pool.tile([64, jlen, FREE], f16)
                src = wmat[2 * j0 : 2 * (j0 + jlen)].rearrange(
                    "(j e) p f -> (e p) j f", e=2
                )
                getattr(nc, eng).dma_start(w_t[:], src)
                for jj in range(jlen):
                    win_of[j0 + jj] = (w_t, jj)

            s_of: dict[int, object] = {}
            n_cp = 0

            def copy_rr(dst, src):
                """Round-robin PSUM->SBUF copies over the two PSUM-capable
                engines (GPSIMD cannot access PSUM)."""
                nonlocal n_cp
                if n_cp % 2 == 0:
                    nc.scalar.copy(dst, src)
                else:
                    nc.vector.tensor_copy(dst, src)
                n_cp += 1

            def s_pair(j):
                """Stacked S = mask @ WxP for the 2 boxes of pair j."""
                if j >= npairs or j in s_of:
                    return
                w_t, jj = win_of[j]
                s_p = psum_s.tile([64, PW], f32)
                for e in range(2):
                    b0 = 32 * e
                    nc.tensor.matmul(
                        s_p[b0 : b0 + 32, :],
                        w_t[b0 : b0 + MW, jj, PW : PW + 32],
                        w_t[b0 : b0 + MW, jj, 0:PW],
                        start=True,
                        stop=True,
                    )
                s_sb = s_pool.tile([64, PW], f16)
                copy_rr(s_sb[:], s_p[:])
                s_of[j] = s_sb

            s_pair(0)
            s_pair(1)
            for k in range(ng):
                s_pair(2 * k + 2)
                s_pair(2 * k + 3)
                stage = stage_pool.tile([PH // 2, GQ, OUT_F], f16, tag="stage")
                for q in range(GQ):
                    j, e = (2 * k + q // 2), (q % 2)
                    w_t, jj = win_of[j]
                    s_sb = s_of[j]
                    b0 = 32 * e
                    p_c = psum_c.tile([PH // 2, OUT_F], f32)
                    nc.tensor.matmul(
                        p_c[:, 0:PW],
                        w_t[b0 : b0 + MW, jj, PW + 32 : PW + 32 + 100],
                        s_sb[b0 : b0 + MW, :],
                        start=True,
                        stop=True,
                    )
                    nc.tensor.matmul(
                        p_c[:, PW : 2 * PW],
                        w_t[b0 : b0 + MW, jj, PW + 132 : PW + 232],
                        s_sb[b0 : b0 + MW, :],
                        start=True,
                        stop=True,
                    )
                    copy_rr(stage[:, q, :], p_c[:])
                win_of.pop(2 * k), win_of.pop(2 * k + 1)
                s_of.pop(2 * k), s_of.pop(2 * k + 1)
                out_ap = out[k * GQ : (k + 1) * GQ].rearrange("c p j -> p c j")
                nc.sync.dma_start(out_ap, stage[:])
    _split_multi_waits(nc)
    return nc


def _box_weight_matrices(det_outs: np.ndarray):
    """Wy [BN, H, MH], Wx [BN, MW, W] (f32) + box corners, reference semantics."""
    det = np.asarray(det_outs, dtype=np.float32).reshape(B * N, 6)
    score = det[:, 5]
    thr = np.float32(50.0) if np.max(score) > 50.0 else np.float32(-100.0)
    valid = score >= thr
    box = np.maximum(det, np.float32(1.0))
    cx, cy, w, h = box[:, 0], box[:, 1], box[:, 2], box[:, 3]
    two = np.float32(2.0)
    xmin = np.clip(np.ceil(cx - w / two).astype(np.int32), 0, W)
    xmax = np.clip(np.ceil(cx + w / two).astype(np.int32), 0, W)
    ymin = np.clip(np.ceil(cy - h / two).astype(np.int32), 0, H)
    ymax = np.clip(np.ceil(cy + h / two).astype(np.int32), 0, H)
    out_h = (ymax - ymin).astype(np.float32)
    out_w = (xmax - xmin).astype(np.float32)
    one = np.float32(1.0)
    sy = np.where(out_h > one, np.float32(MH - 1) / np.maximum(out_h - one, one),
                  np.float32(0.0)).astype(np.float32)
    sx = np.where(out_w > one, np.float32(MW - 1) / np.maximum(out_w - one, one),
                  np.float32(0.0)).astype(np.float32)

    ys = np.arange(H, dtype=np.float32)
    xs = np.arange(W, dtype=np.float32)
    src_y = (ys[None, :] - ymin[:, None].astype(np.float32)) * sy[:, None]
    src_x = (xs[None, :] - xmin[:, None].astype(np.float32)) * sx[:, None]
    src_y = np.clip(src_y, np.float32(0.0), np.float32(MH - 1)).astype(np.float32)
    src_x = np.clip(src_x, np.float32(0.0), np.float32(MW - 1)).astype(np.float32)

    y0 = np.floor(src_y).astype(np.int32)
    y1 = np.minimum(y0 + 1, MH - 1)
    wy = (src_y - y0.astype(np.float32)).astype(np.float32)
    x0 = np.floor(src_x).astype(np.int32)
    x1 = np.minimum(x0 + 1, MW - 1)
    wx = (src_x - x0.astype(np.float32)).astype(np.float32)

    keep_y = ((ys[None, :] >= ymin[:, None].astype(np.float32))
              & (ys[None, :] < ymax[:, None].astype(np.float32))
              & valid[:, None]).astype(np.float32)
    keep_x = ((xs[None, :] >= xmin[:, None].astype(np.float32))
              & (xs[None, :] < xmax[:, None].astype(np.float32))).astype(np.float32)

    m = np.arange(MH, dtype=np.int32)
    Wy = ((m[None, None, :] == y0[:, :, None]) * (one - wy[:, :, None])
          + (m[None, None, :] == y1[:, :, None]) * wy[:, :, None]).astype(np.float32)
    Wy *= keep_y[:, :, None]
    Wx = ((m[None, :, None] == x0[:, None, :]) * (one - wx[:, None, :])
          + (m[None, :, None] == x1[:, None, :]) * wx[:, None, :]).astype(np.float32)
    Wx *= keep_x[:, None, :]
    return Wy, Wx, xmin, ymin


def prepare_in_maps(det_outs: np.ndarray, ins_outs: np.ndarray,
                    cpc: int = CPC, n_cores: int = N_CORES):
    BN = B * N
    Wy, Wx, xmin, ymin = _box_weight_matrices(det_outs)
    # per-box patch slices (every nonzero row/col of Wy/Wx lies inside the
    # 200-wide window starting at ymin/xmin)
    Wy_pad = np.zeros((BN, H + PH, MH), np.float32)
    Wy_pad[:, :H] = Wy
    Wx_pad = np.zeros((BN, MW, W + PW), np.float32)
    Wx_pad[:, :, :W] = Wx
    idx = np.arange(BN)
    WyP = Wy_pad[idx[:, None], ymin[:, None] + np.arange(PH)[None, :]]  # [BN,PH,MH]
    WxP = Wx_pad[idx[:, None], :, xmin[:, None] + np.arange(PW)[None, :]]  # [BN,PW,MW]
    WxP = np.swapaxes(WxP, 1, 2)  # [BN, MW, PW]
    maskT = np.swapaxes(np.asarray(ins_outs, np.float32).reshape(BN, MH, MW), 1, 2)
    wyT = np.swapaxes(WyP, 1, 2)  # [BN, MH, PH]

    wmat = np.zeros((BN, 32, FREE), np.float16)
    wmat[:, :MW, 0:PW] = WxP
    wmat[:, :MW, PW : PW + MH] = maskT
    wmat[:, :MH, PW + 32 : PW + 132] = wyT[:, :, 0::2]
    wmat[:, :MH, PW + 132 : PW + 232] = wyT[:, :, 1::2]
    in_maps = [{"wmat": np.ascontiguousarray(wmat[i * cpc : (i + 1) * cpc])}
               for i in range(n_cores)]
    return in_maps, xmin, ymin


def assemble(core_outs: list, xmin: np.ndarray, ymin: np.ndarray) -> np.ndarray:
    """Paste per-box patches into zero canvases (the unshard step)."""
    full = np.zeros((B * N, H, W), np.float32)
    for k in range(N_CORES):
        patches = np.asarray(core_outs[k], np.float32)  # [cpc, 100, 400]
        for c in range(CPC):
            i = k * CPC + c
            y0, x0 = int(ymin[i]), int(xmin[i])
            vh = min(PH, H - y0)
            vw = min(PW, W - x0)
            if vh > 0 and vw > 0:
                full[i, y0 : y0 + vh, x0 : x0 + vw] = (
                    patches[c].reshape(PH, PW)[:vh, :vw]
                )
    return full.reshape(B, N, H, W)


def kernel(images: np.ndarray, det_outs: np.ndarray, ins_outs: np.ndarray) -> np.ndarray:
    nc = build_nc()
    in_maps, xmin, ymin = prepare_in_maps(det_outs, ins_outs)
    res = run_bass_kernel_spmd(nc, in_maps, list(range(N_CORES)))
    return assemble([res.results[k]["out"] for k in range(N_CORES)], xmin, ymin)
